# revision 10
# baseline (speedup 1.0000x reference)
"""Trainium2 Bass kernel for nn_BaseQuantizer (VQ codebook quantizer).

Strategy (data-parallel over batch, 1 batch row per NeuronCore, 8 cores):
  scores[tok, n] = x[tok] . cb[:, n]  (argmax of  s - 0.5*|cb_n|^2  ==  argmin dist)
  Codebook stored in SBUF as bf16 hi + bf16 lo; exact-fp32-quality scores via
  3 full-rate bf16 matmul passes (hi*hi + hi*lo + lo*hi).  Argmax via DVE
  Max8/MaxIndex directly on PSUM pieces.  Dequantize via gpsimd ap_gather from
  the SBUF-resident codebook (hi+lo reconstruct).  Commitment/codebook loss SSE
  accumulated on-device.  Host does only the cross-shard reductions: concat,
  bincount of device indices, EMA, loss normalization.
"""
import numpy as np

import concourse.bass as bass
import concourse.tile as tile
from concourse import bacc, mybir
from concourse import bass_utils

F32 = mybir.dt.float32
U16 = mybir.dt.uint16
I16 = mybir.dt.int16
U32 = mybir.dt.uint32
U8 = mybir.dt.uint8
BF16 = mybir.dt.bfloat16
AF = mybir.ActivationFunctionType
ALU = mybir.AluOpType

B, L, D, N = 8, 2048, 512, 8192
KC = D // 128            # 4 contraction chunks of 128
NTILES = L // 128        # 16 token tiles per core
ALPHA = 0.95

# argmax pieces in PSUM: 4 pieces x 2048 scores (4 banks each... 2048*4B=8KB=4 banks)
NPIECE = 4
PIECE = N // NPIECE      # 2048


def build_nc():
    nc = bacc.Bacc("TRN2", target_bir_lowering=False, debug=False,
                   enable_asserts=False, num_devices=8)

    # ---------------- DRAM ----------------
    x_d = nc.dram_tensor("x", [L, D], F32, kind="ExternalInput").ap()
    cb_d = nc.dram_tensor("cb", [D, N], F32, kind="ExternalInput").ap()
    nbias_d = nc.dram_tensor("nbias", [N], F32, kind="ExternalInput").ap()  # -0.5*|cb_n|^2
    nbh_d = nc.dram_tensor("nbh_scratch", [3, N], BF16, kind="Internal").ap()

    xst_d = nc.dram_tensor("xst", [L, D], F32, kind="ExternalOutput").ap()
    idx_d = nc.dram_tensor("idx", [L], U16, kind="ExternalOutput").ap()
    sse_d = nc.dram_tensor("sse", [1, 1], F32, kind="ExternalOutput").ap()

    # ---------------- SBUF (persistent) ----------------
    cb_hl = nc.alloc_sbuf_tensor("cb_hl", [128, KC, N, 2], BF16).ap()  # d = c*128+p; [...,0]=hi [...,1]=lo
    ident = nc.alloc_sbuf_tensor("ident", [128, 128], F32).ap()
    iota_r = nc.alloc_sbuf_tensor("iota_r", [128, 128], F32).ap()
    iota_c = nc.alloc_sbuf_tensor("iota_c", [128, 1], F32).ap()
    ones_col = nc.alloc_sbuf_tensor("ones_col", [128, 1], F32).ap()
    ones_k3 = nc.alloc_sbuf_tensor("ones_k3", [3, 128], BF16).ap()
    bias_hml = nc.alloc_sbuf_tensor("bias_hml", [3, N], BF16).ap()
    nb32 = nc.alloc_sbuf_tensor("nb32", [128, N // 128], F32).ap()
    nbtmp = nc.alloc_sbuf_tensor("nbtmp", [128, N // 128, 3], BF16).ap()
    nbr = nc.alloc_sbuf_tensor("nbr", [128, N // 128], F32).ap()
    ssev = nc.alloc_sbuf_tensor("ssev", [128, NTILES], F32).ap()
    sse_sb = nc.alloc_sbuf_tensor("sse_sb", [1, 1], F32).ap()

    # ---------------- PSUM ----------------
    # two physical regions of 4 banks; the 4 logical score pieces alternate
    # between them; transposes + sse reduce time-share the same regions
    # (scheduler serializes via WAR/WAW deps).
    psA = nc.alloc_psum_tensor("psA", [128, PIECE], F32).ap()
    psB = nc.alloc_psum_tensor("psB", [128, PIECE], F32).ap()
    ps = [psA, psB, psA, psB]

    with tile.TileContext(nc) as tc:
        # ---- constants ----
        nc.gpsimd.iota(iota_r[:], pattern=[[1, 128]], base=0, channel_multiplier=0,
                       allow_small_or_imprecise_dtypes=True)
        nc.gpsimd.iota(iota_c[:], pattern=[[0, 1]], base=0, channel_multiplier=1,
                       allow_small_or_imprecise_dtypes=True)
        nc.vector.tensor_scalar(ident[:], iota_r[:], iota_c[:], None, ALU.is_equal)
        nc.vector.memset(ones_col[:], 1.0)
        nc.gpsimd.memset(ones_k3[:], 1.0)

        # ---- 3-way bf16 split of nbias (wide layout, then bounce to [3, N]) ----
        nc.sync.dma_start(nb32[:], nbias_d.rearrange("(p j) -> p j", p=128))
        nc.scalar.copy(nbtmp[:, :, 0], nb32[:])                      # hi
        nc.vector.tensor_sub(nbr[:], nb32[:], nbtmp[:, :, 0])        # r1 = b - hi (f32)
        nc.scalar.copy(nbtmp[:, :, 1], nbr[:])                       # mid
        nc.vector.tensor_sub(nbtmp[:, :, 2], nbr[:], nbtmp[:, :, 1])  # lo (bf16)
        with nc.allow_non_contiguous_dma(reason="small bias bounce"):
            for r in range(3):
                nc.sync.dma_start(nbh_d[r].rearrange("(p j) -> p j", p=128), nbtmp[:, :, r])
        nc.sync.dma_start(bias_hml[:], nbh_d[:])

        # ---- load + split codebook into bf16 hi/lo ----
        cb_r = cb_d.rearrange("(c p) n -> p c n", c=KC, p=128)
        with tc.tile_pool(name="cbstage", bufs=2) as cbst_pool:
            for c in range(KC):
                for h in range(4):
                    sl = slice(h * 2048, (h + 1) * 2048)
                    cbst = cbst_pool.tile([128, 2048], F32, tag="cbst")
                    nc.sync.dma_start(cbst[:], cb_r[:, c, sl])
                    nc.scalar.copy(cb_hl[:, c, sl, 0], cbst[:])                      # f32 -> bf16 (hi)
                    nc.vector.tensor_sub(cb_hl[:, c, sl, 1], cbst[:], cb_hl[:, c, sl, 0])  # f32 - bf16 -> bf16 (lo)

        # ---- tile pools for the main loop ----
        with (
            tc.tile_pool(name="xrow", bufs=3) as xrow_pool,
            tc.tile_pool(name="xt", bufs=3) as xt_pool,
            tc.tile_pool(name="small", bufs=4) as small_pool,
            tc.tile_pool(name="gath", bufs=3) as gath_pool,
            tc.tile_pool(name="outp", bufs=3) as out_pool,
        ):
            for t in range(NTILES):
                tok = slice(t * 128, (t + 1) * 128)
                # ---- load x tile and transpose to [d, tok] ----
                xr = xrow_pool.tile([128, D], F32, tag="xr")
                nc.sync.dma_start(xr[:], x_d[tok, :])
                xt32 = xt_pool.tile([128, KC, 128], F32, tag="xt32")
                for c in range(KC):
                    pst = psA[:, c * 128:(c + 1) * 128]
                    nc.tensor.transpose(pst, xr[:, c * 128:(c + 1) * 128], ident[:])
                    nc.scalar.copy(xt32[:, c, :], pst)
                xhi = xt_pool.tile([128, KC, 128], BF16, tag="xhi")
                xlo = xt_pool.tile([128, KC, 128], BF16, tag="xlo")
                nc.scalar.copy(xhi[:], xt32[:])
                nc.vector.tensor_sub(xlo[:], xt32[:], xhi[:])

                # ---- score matmuls (3 bf16 passes + bias seed) and per-piece
                #      argmax scans, interleaved so each piece is scanned
                #      before its psum region is reused by piece pc+2 ----
                mx8 = small_pool.tile([128, NPIECE, 8], F32, tag="mx8")
                ix8 = small_pool.tile([128, NPIECE, 8], U32, tag="ix8")
                for pc in range(NPIECE):
                    for nci in range(PIECE // 512):
                        n0 = pc * PIECE + nci * 512
                        outap = ps[pc][:, nci * 512:(nci + 1) * 512]  # ps[pc] is psA/psB alternating
                        nc.tensor.matmul(outap, ones_k3[:], bias_hml[:, n0:n0 + 512],
                                         start=True, stop=False)
                        for kc in range(KC):
                            for (lh, hl) in ((xhi, 0), (xhi, 1), (xlo, 0)):
                                nc.tensor.matmul(
                                    outap,
                                    lh[:, kc, :],
                                    cb_hl[:, kc, n0:n0 + 512, hl],
                                    start=False, stop=(kc == KC - 1 and hl == 0 and lh is xlo),
                                )
                    nc.vector.max(mx8[:, pc, :], ps[pc][:])
                    nc.vector.max_index(ix8[:, pc, :], mx8[:, pc, :], ps[pc][:])
                # merge select-tree (favor lower piece on ties -> first occurrence)
                mxw = small_pool.tile([128, 4], F32, tag="mxw")    # work: pairwise maxima
                ixw = small_pool.tile([128, 4], F32, tag="ixw")    # indices as f32 (exact <= 2^24)
                ixf = small_pool.tile([128, NPIECE], F32, tag="ixf")
                msk = small_pool.tile([128, 2], U8, tag="msk")
                for pc in range(NPIECE):
                    # u32 idx -> f32, add piece base
                    nc.vector.tensor_scalar(ixf[:, pc:pc + 1], ix8[:, pc, 0:1], float(pc * PIECE), None, ALU.add)
                # level 1: (0 vs 1) and (2 vs 3)
                for h in range(2):
                    a, b_ = 2 * h, 2 * h + 1
                    nc.vector.tensor_single_scalar(msk[:, h:h + 1], mx8[:, a, 0:1], mx8[:, b_, 0:1], ALU.is_ge)
                    nc.vector.tensor_max(mxw[:, h:h + 1], mx8[:, a, 0:1], mx8[:, b_, 0:1])
                    nc.vector.select(ixw[:, h:h + 1], msk[:, h:h + 1], ixf[:, a:a + 1], ixf[:, b_:b_ + 1])
                # level 2
                nc.vector.tensor_single_scalar(msk[:, 0:1], mxw[:, 0:1], mxw[:, 1:2], ALU.is_ge)
                nc.vector.select(ixw[:, 2:3], msk[:, 0:1], ixw[:, 0:1], ixw[:, 1:2])
                idx16 = small_pool.tile([128, 1], U16, tag="idx16")
                nc.vector.tensor_copy(idx16[:], ixw[:, 2:3])       # f32 -> u16

                # ---- write idx (token order) + read back wrapped for gather ----
                nc.sync.dma_start(idx_d.rearrange("(tt p) -> p tt", p=128)[:, t:t + 1], idx16[:])
                idxw = small_pool.tile([128, 8], I16, tag="idxw")
                with nc.allow_non_contiguous_dma(reason="256B wrapped idx load"):
                    for g in range(8):
                        nc.sync.dma_start(
                            idxw[16 * g:16 * (g + 1), :],
                            idx_d[t * 128:(t + 1) * 128].rearrange("(m k) -> k m", k=16).bitcast(I16))

                # ---- gather hi/lo codebook rows, reconstruct xq (f32) ----
                ghl = gath_pool.tile([128, KC, 128, 2], BF16, tag="ghl")
                for c in range(KC):
                    nc.gpsimd.ap_gather(
                        ghl[:, c, :, :],
                        cb_hl[:, c, :, :],
                        idxw[:], channels=128, num_elems=N, d=2, num_idxs=128)
                xq32 = gath_pool.tile([128, KC, 128], F32, tag="xq32")
                nc.gpsimd.tensor_add(xq32[:], ghl[:, :, :, 0], ghl[:, :, :, 1])

                # ---- transpose xq -> [tok, d], write x_st ----
                xst = out_pool.tile([128, D], F32, tag="xst")
                for c in range(KC):
                    pst = psB[:, c * 128:(c + 1) * 128]
                    nc.tensor.transpose(pst, xq32[:, c, :], ident[:])
                    nc.scalar.copy(xst[:, c * 128:(c + 1) * 128], pst)
                nc.sync.dma_start(xst_d[tok, :], xst[:])

                # ---- SSE: in-place diff + square-accumulate ----
                nc.gpsimd.tensor_sub(xq32[:], xq32[:], xt32[:])
                nc.scalar.activation(xq32[:], xq32[:], AF.Square, accum_out=ssev[:, t:t + 1])

            # ---- final SSE reduction: sum ssev over tiles then partitions ----
            ssetot = small_pool.tile([128, 1], F32, tag="ssetot")
            nc.vector.tensor_reduce(ssetot[:], ssev[:], axis=mybir.AxisListType.X, op=ALU.add)
            nc.tensor.matmul(psA[0:1, 0:1], ssetot[:], ones_col[:], start=True, stop=True)
            nc.scalar.copy(sse_sb[:], psA[0:1, 0:1])
            nc.sync.dma_start(sse_d[:], sse_sb[:])

    nc.compile()
    return nc


_NC_CACHE = None


def _get_nc():
    global _NC_CACHE
    if _NC_CACHE is None:
        _NC_CACHE = build_nc()
    return _NC_CACHE


def kernel(x_in: np.ndarray, codebook: np.ndarray, cluster_frequency: np.ndarray):
    assert x_in.shape == (B, L, D) and codebook.shape == (D, N)
    nc = _get_nc()
    x_in = np.ascontiguousarray(x_in, dtype=np.float32)
    codebook = np.ascontiguousarray(codebook, dtype=np.float32)

    nbias = (-0.5 * (codebook.astype(np.float64) ** 2).sum(0)).astype(np.float32)
    in_maps = [{"x": x_in[b], "cb": codebook, "nbias": nbias} for b in range(B)]
    res = bass_utils.run_bass_kernel_spmd(nc, in_maps, core_ids=list(range(B)))

    xst = np.stack([res.results[b]["xst"] for b in range(B)])           # [B, L, D]
    idx = np.stack([res.results[b]["idx"].astype(np.int32) for b in range(B)])  # [B, L]
    sse = np.array([res.results[b]["sse"].ravel()[0] for b in range(B)])

    # host-side cross-shard reductions (per sharding strategy)
    inner_loss = np.float32(2.0 * (np.float64(sse.sum()) / (B * L * D)))
    counts = np.bincount(idx.reshape(-1), minlength=N).astype(np.float32)
    new_cf = (np.float32(ALPHA) * cluster_frequency.astype(np.float32)
              + np.float32(1.0 - ALPHA) * counts)
    return xst, idx, inner_loss, new_cf
